# revision 27
# baseline (speedup 1.0000x reference)
"""Distributed Trainium2 Bass kernel for fused LayerNorm + causal multi-head
attention + output projection (B=2, T=2048, DIM=1024, H=16, D=64) on 8 cores.

Structure (v5):
  - Interleaved token ownership: core c owns tokens [256c:256c+256) of BOTH
    batches, so input/output redistribution splits per batch and pipelines.
  - Input side: three A2As ordered KQ-b0 (1MB, fires right after the
    firmware barrier), V (producer-transposed, softmax-ones columns
    embedded so the consumer DMAs straight into vnat layout), KQ-b1.
    Batch-0 attention starts while KQ-b1 is still in flight.
  - x shipped as bf16; weight loads split across many DMA engines in
    consumption order (wk interleaved with x, then wq, wv).
  - Attention: causal-trimmed score/exp/PV tiles, exp over both heads in
    one ACT instruction, diagonal-only masking, late normalization via a
    broadcast matmul, PV backlog of 2 to decouple PE from ACT; batch-0's
    first (largest) qc defers its first 8 PVs so scores/exp stream while
    the V collective lands.
  - Output: per-batch A2As; batch-0's projection matmuls are fed into
    batch-1 attention's PE bubbles (replacing keep-warm dummies with
    real work), batch-1 projection + split output DMAs form the tail.

Compute dtype: bf16 matmuls with fp32 PSUM accumulation.
LN affine params and the 1/sqrt(D) score scale are folded into the QKV
weights on the host.
"""
import os
import sys
import types
import numpy as np
import ml_dtypes

# ---------------------------------------------------------------- constants
B, T, DIM, D = 2, 2048, 1024, 64
H = DIM // D            # 16 heads
NC = 8                  # cores
TOK = B * T             # 4096 tokens
TPC = TOK // NC         # 512 tokens per core (256 per batch)
CH = TPC // 2           # 256-token per-batch chunk
KT8 = DIM // 128        # 8 contraction tiles
EPS = 1e-5

TRACE = bool(int(os.environ.get("BASS_KERNEL_TRACE", "0")))
DUM_LN = int(os.environ.get("DUM_LN", "40"))      # transposes during startup/LN
DUM_W1 = int(os.environ.get("DUM_W1", "140"))      # 512-wide, while waiting for qT
DUMF_W = int(os.environ.get("DUMF_W", "128"))     # width of b0 bubble fillers
DUM_TAIL = int(os.environ.get("DUM_TAIL", "36"))  # while A2A#5 flies
GPB = bool(int(os.environ.get("GPB", "0")))       # gpsimd partition_broadcast
SHARED_CC = bool(int(os.environ.get("SHARED_CC", "0")))

BF16_NP = ml_dtypes.bfloat16


def _ensure_ntff_hook():
    """The agent image lacks antenv.axon_hooks; recreate it so trace=True works."""
    if "antenv.axon_hooks" not in sys.modules:
        mod = types.ModuleType("antenv.axon_hooks")
        mod._hook = None
        def set_axon_ntff_profile_hook(h):
            mod._hook = h
        def get_axon_ntff_profile_hook():
            return mod._hook
        mod.set_axon_ntff_profile_hook = set_axon_ntff_profile_hook
        mod.get_axon_ntff_profile_hook = get_axon_ntff_profile_hook
        sys.modules["antenv.axon_hooks"] = mod
    m = sys.modules["antenv.axon_hooks"]
    if m.get_axon_ntff_profile_hook() is None:
        try:
            from trn_agent_boot.trn_boot import _ntff_profile_via_ctypes
            m.set_axon_ntff_profile_hook(
                _ntff_profile_via_ctypes("/opt/axon/libaxon_pjrt.so"))
        except Exception:
            pass


def build_graph():
    import concourse.bass as bass
    import concourse.bacc as bacc
    import concourse.tile as tile
    import concourse.mybir as mybir

    dt = mybir.dt
    F32, BF16, F8 = dt.float32, dt.bfloat16, dt.float8e4
    AF = mybir.ActivationFunctionType
    ALU = mybir.AluOpType
    RG = [list(range(NC))]
    SH = "Shared" if SHARED_CC else "Local"

    nc = bacc.Bacc(None, target_bir_lowering=False, debug=False, num_devices=NC)

    # ------------------------------------------------------------ I/O
    x_in = nc.dram_tensor("x_c", [TPC, DIM], BF16, kind="ExternalInput")
    wk_in = nc.dram_tensor("wk", [128, KT8 * 1024], BF16, kind="ExternalInput")
    wq_in = nc.dram_tensor("wq", [128, KT8 * 1024], BF16, kind="ExternalInput")
    wv_in = nc.dram_tensor("wv", [128, KT8 * 1024], BF16, kind="ExternalInput")
    bk_in = nc.dram_tensor("bk", [128, 8], F32, kind="ExternalInput")
    bq_in = nc.dram_tensor("bq", [128, 8], F32, kind="ExternalInput")
    bv_in = nc.dram_tensor("bv", [128, 8], F32, kind="ExternalInput")
    pwt_in = nc.dram_tensor("pwt", [128, KT8 * DIM], BF16, kind="ExternalInput")
    pb_in = nc.dram_tensor("pb", [1, DIM], BF16, kind="ExternalInput")
    idn_in = nc.dram_tensor("idn", [128, 128], BF16, kind="ExternalInput")
    ones_in = nc.dram_tensor("ones_r", [1, 128], BF16, kind="ExternalInput")
    emat_in = nc.dram_tensor("emat", [33, 128], BF16, kind="ExternalInput")
    out_dram = nc.dram_tensor("out_c", [TPC, DIM], F32, kind="ExternalOutput")

    with tile.TileContext(nc) as tc:
        with (
            tc.tile_pool(name="persist", bufs=1) as pers,
            tc.tile_pool(name="dram", bufs=1, space="DRAM") as dram,
        ):
            # ---------------- DRAM bounce buffers ----------------
            # A2A#1/#3: block c = [K 128 | Q 128] for dest c, one per batch
            kq_in = [dram.tile([NC * 256, CH], BF16, name=f"kq_in{b}")
                     for b in range(B)]
            kq_out = [dram.tile([NC * 256, CH], BF16, name=f"kq_out{b}")
                      for b in range(B)]
            # A2A#2: V, block c = [128 local tokens, 4 x (A 64|1|B 64|1)]
            v_in = dram.tile([NC * 128, 520], BF16)
            v_out = dram.tile([NC * 128, 520], BF16)
            ao_in = [dram.tile([NC * 128, CH], BF16, name=f"ao_in{b}")
                     for b in range(B)]
            ao_out = [dram.tile([NC * 128, CH], BF16, addr_space=SH,
                                name=f"ao_out{b}") for b in range(B)]

            if GPB:
                from concourse import library_config
                nc.gpsimd.load_library(library_config.attn)

            # idn first: transposes + dummies need it early; it is tiny
            idn_sb = pers.tile([128, 128], BF16)
            nc.sync.dma_start(idn_sb[:], idn_in[:])

            # x tiles FIRST (LN critical path), then QKV weights in
            # consumption order (wk, wq, wv) split across many DMA engines.
            xts = []
            with tc.tile_pool(name="ln_x", bufs=1) as lnx:
                wk_sb = pers.tile([128, KT8 * 1024], BF16)
                for t in range(4):
                    xt = lnx.tile([128, DIM], BF16, tag=f"xt{t}", name=f"xt{t}")
                    for hh in range(2):
                        nc.sync.dma_start(
                            xt[:, 512 * hh:512 * (hh + 1)],
                            x_in[128 * t:128 * (t + 1), 512 * hh:512 * (hh + 1)])
                    xts.append(xt)
                    for i in (2 * t, 2 * t + 1):
                        nc.sync.dma_start(wk_sb[:, 1024 * i:1024 * (i + 1)],
                                          wk_in[:, 1024 * i:1024 * (i + 1)])
                wq_sb = pers.tile([128, KT8 * 1024], BF16)
                for i in range(8):
                    nc.sync.dma_start(wq_sb[:, 1024 * i:1024 * (i + 1)],
                                      wq_in[:, 1024 * i:1024 * (i + 1)])
                wv_sb = pers.tile([128, KT8 * 1024], BF16)
                for i in range(8):
                    nc.sync.dma_start(wv_sb[:, 1024 * i:1024 * (i + 1)],
                                      wv_in[:, 1024 * i:1024 * (i + 1)])
                bk_sb = pers.tile([128, 8], F32)
                nc.sync.dma_start(bk_sb[:], bk_in[:])
                bq_sb = pers.tile([128, 8], F32)
                nc.sync.dma_start(bq_sb[:], bq_in[:])
                bv_sb = pers.tile([128, 8], F32)
                nc.sync.dma_start(bv_sb[:], bv_in[:])

                # ============= P1: LayerNorm (token slice, natural) ========
                xn_sb = pers.tile([128, 4 * DIM], BF16)
                with tc.tile_pool(name="ln", bufs=4) as lnp:
                    if DUM_LN:
                        with tc.tile_pool(name="ps_dln", bufs=1,
                                          space="PSUM") as psdl:
                            dln = psdl.tile([128, 128], BF16, tag="dln")
                            for i in range(DUM_LN):
                                nc.tensor.transpose(dln[:], idn_sb[:], idn_sb[:])
                    for t in range(4):
                        xt = xts[t]
                        stats = lnp.tile([128, 12], F32, tag="stats")
                        nc.vector.bn_stats(stats[:, 0:6], xt[:, 0:512])
                        nc.vector.bn_stats(stats[:, 6:12], xt[:, 512:1024])
                        mv = lnp.tile([128, 2], F32, tag="mv")
                        nc.vector.bn_aggr(mv[:], stats[:])
                        vareps = lnp.tile([128, 1], F32, tag="vareps")
                        nc.vector.tensor_scalar(vareps[:], mv[:, 1:2], 1.0, EPS,
                                                op0=ALU.mult, op1=ALU.add)
                        nmu = lnp.tile([128, 1], F32, tag="nmu")
                        nc.vector.tensor_scalar_mul(nmu[:], mv[:, 0:1], -1.0)
                        std = lnp.tile([128, 1], F32, tag="std")
                        nc.scalar.activation(std[:], vareps[:], AF.Sqrt)
                        rstd = lnp.tile([128, 1], F32, tag="rstd")
                        nc.vector.reciprocal(rstd[:], std[:])
                        nmr = lnp.tile([128, 1], F32, tag="nmr")
                        nc.vector.scalar_tensor_tensor(
                            nmr[:], nmu[:], 1.0, rstd[:],
                            op0=ALU.mult, op1=ALU.mult)
                        nc.scalar.activation(
                            xn_sb[:, DIM * t:DIM * (t + 1)], xt[:],
                            AF.Identity, bias=nmr[:], scale=rstd[:])

            # preload the EXP activation table while nothing else uses ACT
            exq = pers.tile([1, 1], F32, name="exq")
            nc.scalar.activation(exq[:], xn_sb[0:1, 0:1], AF.Exp)

            # ================= P2: transpose xn -> xnT =====================
            xnT_sb = pers.tile([128, KT8 * TPC], BF16)  # [dim-tile part, k*512+t]
            with tc.tile_pool(name="ps_tr", bufs=6, space="PSUM") as pstr:
                for t in range(4):
                    for k in range(KT8):
                        trp = pstr.tile([128, 128], BF16, tag="tr")
                        nc.tensor.transpose(
                            trp[:], xn_sb[:, DIM * t + 128 * k: DIM * t + 128 * (k + 1)],
                            idn_sb[:])
                        nc.vector.tensor_copy(
                            xnT_sb[:, TPC * k + 128 * t: TPC * k + 128 * (t + 1)],
                            trp[:])

            # ================= P3: K, Q, V on own tokens + 3 A2As ==========
            wk3 = wk_sb[:].rearrange("p (k r) -> p k r", r=1024)
            wq3 = wq_sb[:].rearrange("p (k r) -> p k r", r=1024)
            wv3 = wv_sb[:].rearrange("p (k r) -> p k r", r=1024)
            with (
                tc.tile_pool(name="ps_qkv", bufs=3, space="PSUM") as psq,
                tc.tile_pool(name="ps_vt", bufs=2, space="PSUM") as psvt,
                tc.tile_pool(name="ps_dum", bufs=1, space="PSUM") as psd,
                tc.tile_pool(name="stg", bufs=4) as stg,
            ):
                def emit_half(w3, g, bias_sb, b, dst, use_act=False):
                    # one K/Q group, batch-b column half only
                    psgf = psq.tile([128, TPC], F32, tag="qg", name="psgf")
                    psg = psgf[:, 0:CH]
                    for k in range(KT8):
                        nc.tensor.matmul(
                            psg, w3[:, k, 128 * g:128 * (g + 1)],
                            xnT_sb[:, TPC * k + CH * b:TPC * k + CH * (b + 1)],
                            start=(k == 0), stop=(k == KT8 - 1))
                    st = stg.tile([128, CH], BF16, tag="st", name="st")
                    if use_act:
                        # Scalar engine: keeps pass-1 staging off the DVE
                        # queue, whose sem-recycle guards wait on the unpack
                        nc.scalar.activation(st[:], psg, AF.Identity,
                                             bias=bias_sb[:, g:g + 1])
                    else:
                        nc.vector.tensor_scalar(
                            st[:], psg, bias_sb[:, g:g + 1], None, op0=ALU.add)
                    nc.sync.dma_start(dst, st[:])

                # pass 0: K+Q batch-0 columns -> A2A#1 fires first
                for g in range(8):
                    emit_half(wk3, g, bk_sb, 0,
                              kq_in[0][256 * g:256 * g + 128, :])
                for g in range(8):
                    emit_half(wq3, g, bq_sb, 0,
                              kq_in[0][256 * g + 128:256 * g + 256, :])
                nc.gpsimd.collective_compute(
                    "AllToAll", ALU.bypass, replica_groups=RG,
                    ins=[kq_in[0][:].opt()], outs=[kq_out[0][:].opt()],
                )
                # V (full tokens): producer-side transpose with ones
                for g in range(8):
                    psg = psq.tile([128, TPC], F32, tag="qg", name="psgv")
                    for k in range(KT8):
                        nc.tensor.matmul(
                            psg[:], wv3[:, k, 128 * g:128 * (g + 1)],
                            xnT_sb[:, TPC * k:TPC * (k + 1)],
                            start=(k == 0), stop=(k == KT8 - 1))
                    st = stg.tile([128, TPC], BF16, tag="st2", name="stv")
                    nc.vector.tensor_scalar(
                        st[:], psg[:], bv_sb[:, g:g + 1], None, op0=ALU.add)
                    vtp = psvt.tile([128, 512], BF16, tag="vtp")
                    for i in range(4):
                        nc.tensor.transpose(
                            vtp[:, 128 * i:128 * (i + 1)],
                            st[:, 128 * i:128 * (i + 1)], idn_sb[:])
                    vts = stg.tile([128, 520], BF16, tag="vts", name="vts")
                    v4s = vts[:].rearrange("p (i a w) -> p i a w", i=4, w=65)
                    nc.vector.memset(v4s[:, :, :, 64:65], 1.0)
                    nc.vector.tensor_copy(
                        v4s[:, :, :, 0:64],
                        vtp[:].rearrange("p (i a w) -> p i a w", i=4, w=64))
                    nc.sync.dma_start(v_in[128 * g:128 * (g + 1), :], vts[:])
                nc.gpsimd.collective_compute(
                    "AllToAll", ALU.bypass, replica_groups=RG,
                    ins=[v_in[:].opt()], outs=[v_out[:].opt()],
                )
                # pass 1: K+Q batch-1 columns (bias on Scalar engine)
                for g in range(8):
                    emit_half(wk3, g, bk_sb, 1,
                              kq_in[1][256 * g:256 * g + 128, :], use_act=True)
                for g in range(8):
                    emit_half(wq3, g, bq_sb, 1,
                              kq_in[1][256 * g + 128:256 * g + 256, :],
                              use_act=True)
                nc.gpsimd.collective_compute(
                    "AllToAll", ALU.bypass, replica_groups=RG,
                    ins=[kq_in[1][:].opt()], outs=[kq_out[1][:].opt()],
                )

                # deferred weight loads (needed only from attention onwards)
                pwt_sb = pers.tile([128, KT8 * DIM], BF16)
                for i in range(4):
                    nc.sync.dma_start(pwt_sb[:, 2048 * i:2048 * (i + 1)],
                                      pwt_in[:, 2048 * i:2048 * (i + 1)])
                pb_sb = pers.tile([1, DIM], BF16)
                nc.sync.dma_start(pb_sb[:], pb_in[:])
                ones_sb = pers.tile([1, 128], BF16)
                nc.sync.dma_start(ones_sb[:], ones_in[:])
                emat_sb = pers.tile([33, 128], BF16)
                nc.sync.dma_start(emat_sb[:], emat_in[:])

                sums_col = pers.tile([33, 512], F32)
                nc.vector.memset(sums_col[:], 1.0)

                # ---- unpack: kT/qT uninterleave, per batch as A2As land
                kqT = pers.tile([128, 2 * TOK], BF16)
                kT_sb = kqT[:, 0:TOK]
                qT_sb = kqT[:, TOK:2 * TOK]
                kqT5 = kqT[:].rearrange("p (s j u t) -> p s j u t",
                                        s=2, j=2, u=NC)
                vnat = []
                for b in range(B):
                    vb = pers.tile([128, 16 * 130], BF16, name=f"vnat{b}")
                    vnat.append(vb)

                v4o = v_out[:].rearrange("(u p) (bb c) -> p u bb c",
                                         p=128, bb=2)

                def unpack_b(b):
                    kq4 = kq_out[b][:].rearrange("(u s p) t -> p u s t",
                                                 p=128, s=2)
                    # both batches run qc3 first: queries live in r=6,7
                    order = (6, 7, 0, 1, 2, 3, 4, 5)
                    for r in order:
                        nc.sync.dma_start(kqT5[:, :, b, r, :],
                                          kq4[:, r, :, :])
                    vn3 = vnat[b][:].rearrange("p (u c) -> p u c", u=NC)
                    for r in range(NC):
                        nc.sync.dma_start(vn3[:, r, :], v4o[:, r, b, :])

                unpack_b(0)

                # keep-warm while waiting for qT
                if DUM_W1:
                    dum = psd.tile([128, 512], F32, tag="dum")
                    for i in range(DUM_W1):
                        nc.tensor.matmul(dum[:], idn_sb[:],
                                         xn_sb[:, 512 * (i % 8):512 * (i % 8) + 512],
                                         start=True, stop=True)

            attnT = pers.tile([128, TOK], BF16)

            # ================= P6: attention + overlapped projection =======
            with (
                tc.tile_pool(name="pt", bufs=10) as ptp,
                tc.tile_pool(name="un", bufs=4) as unp,
                tc.tile_pool(name="ps_s", bufs=2, space="PSUM") as pss,
                tc.tile_pool(name="ps_pv", bufs=2, space="PSUM") as psp,
                tc.tile_pool(name="ps_pj", bufs=2, space="PSUM") as pspj,
                tc.tile_pool(name="sm", bufs=2) as smp,
                tc.tile_pool(name="pjx", bufs=1) as pjx,
                tc.tile_pool(name="pjo", bufs=2) as pjo,
            ):
                pwt3 = pwt_sb[:].rearrange("p (k o) -> p k o", o=DIM)

                def dum_slot(n=1, w=DUMF_W):
                    # keep-warm into the (currently free) proj psum pool
                    for _ in range(n):
                        dt_ = pspj.tile([128, 512], F32, tag="pacc", name="dumpj")
                        nc.tensor.matmul(dt_[:, 0:w], idn_sb[:],
                                         xn_sb[:, 0:w], start=True, stop=True)

                def make_proj_slots(b, aTb):
                    """Thunk list: projection of batch b fed one op per slot."""
                    slots = []
                    state = {}

                    def mm(m, ck):
                        def f():
                            if ck == 0:
                                state["acc"] = pspj.tile([128, 512], F32, tag="pacc", name="pacc")
                            tt, h = m // 2, m % 2
                            nc.tensor.matmul(
                                state["acc"][:],
                                aTb[:, 256 * ck + 128 * tt:256 * ck + 128 * (tt + 1)],
                                pwt3[:, ck, 512 * h:512 * (h + 1)],
                                start=(ck == 0), stop=False)
                        return f

                    def fin(m):
                        def f():
                            tt, h = m // 2, m % 2
                            acc = state["acc"]
                            nc.tensor.matmul(acc[:], ones_sb[0:1, :],
                                             pb_sb[:, 512 * h:512 * (h + 1)],
                                             start=False, stop=True)
                            ot = pjo.tile([128, 512], F32, tag="ot", name="ot")
                            nc.vector.tensor_copy(ot[:], acc[:])
                            r0 = 256 * b + 128 * tt
                            nc.sync.dma_start(
                                out_dram[r0:r0 + 64, 512 * h:512 * (h + 1)],
                                ot[0:64, :])
                            nc.sync.dma_start(
                                out_dram[r0 + 64:r0 + 128, 512 * h:512 * (h + 1)],
                                ot[64:128, :])
                        return f

                    for m in range(4):
                        for ck in range(KT8):
                            slots.append(mm(m, ck))
                        slots.append(fin(m))
                    return slots

                def emit_attention(b, slots=None, slot_from_qc=99,
                                   after_qc=None, defer_kp=0, qc_desc=False):
                    def slot(n):
                        for _ in range(n):
                            if slots and qi >= slot_from_qc:
                                if len(slots):
                                    slots.pop(0)()
                            elif DUMF_W:
                                dum_slot(1)

                    def emit_pv(pkp, pp, pc0, stop):
                        nc.tensor.matmul(pvA[:, pc0:512],
                                         vnat[b][:, 130 * pkp:130 * pkp + 65],
                                         pp[:, pc0:512],
                                         start=(pkp == 0), stop=stop)
                        nc.tensor.matmul(pvB[:, pc0:512],
                                         vnat[b][:, 130 * pkp + 65:130 * pkp + 130],
                                         pp[:, 512 + pc0:1024],
                                         start=(pkp == 0), stop=stop)

                    for qi, qc in enumerate((3, 2, 1, 0)
                                            if (defer_kp or qc_desc) else
                                            (0, 1, 2, 3)):
                        q0 = b * T + 512 * qc
                        pvA = psp.tile([65, 512], F32, tag="pv")
                        pvB = psp.tile([65, 512], F32, tag="pv")
                        nkp = 4 * qc + 4
                        D = defer_kp if qi == 0 else 0
                        pvq = []
                        for kp in range(nkp):
                            k0 = b * T + 128 * kp
                            c0 = max(0, 128 * (kp - 4 * qc))
                            sAB = pss.tile([128, 1024], F32, tag="s")
                            nc.tensor.matmul(sAB[:, c0:512],
                                             kT_sb[0:64, k0:k0 + 128],
                                             qT_sb[0:64, q0 + c0:q0 + 512],
                                             start=True, stop=True)
                            nc.tensor.matmul(sAB[:, 512 + c0:1024],
                                             kT_sb[64:128, k0:k0 + 128],
                                             qT_sb[64:128, q0 + c0:q0 + 512],
                                             start=True, stop=True)
                            ne = 0
                            nmax = 3 if len(pvq) > 4 else 2
                            while kp >= D and len(pvq) > 2 and ne < nmax:
                                pk, pp_, pc = pvq.pop(0)
                                emit_pv(pk, pp_, pc, False)
                                ne += 1
                            if ne == 0:
                                slot(2)
                            pAB = ptp.tile([128, 1024], BF16, tag="pA")
                            if c0 == 0:
                                nc.scalar.activation(pAB[:], sAB[:], AF.Exp)
                            else:
                                s3 = sAB[:].rearrange("p (j c) -> p j c", j=2)
                                p3 = pAB[:].rearrange("p (j c) -> p j c", j=2)
                                nc.scalar.activation(p3[:, :, c0:512],
                                                     s3[:, :, c0:512], AF.Exp)
                            if kp >= 4 * qc:
                                p3 = pAB[:].rearrange("p (j c) -> p j c", j=2)
                                nc.gpsimd.affine_select(
                                    p3[:, :, c0:c0 + 128], p3[:, :, c0:c0 + 128],
                                    pattern=[[0, 2], [1, 128]],
                                    compare_op=ALU.is_ge, fill=0.0,
                                    base=0, channel_multiplier=-1)
                            if kp + 1 < nkp:
                                slot(1)
                            pvq.append((kp, pAB, c0))
                        while pvq:
                            pk, pp_, pc = pvq.pop(0)
                            emit_pv(pk, pp_, pc, stop=(not pvq))
                        # late normalization: copy PV out of PSUM immediately
                        u = unp.tile([128, 512], BF16, tag="u")
                        nc.vector.tensor_copy(sums_col[0:1, :], pvA[64:65, :])
                        nc.vector.tensor_copy(sums_col[32:33, :], pvB[64:65, :])
                        nc.vector.tensor_copy(u[0:64, :], pvA[0:64, :])
                        nc.vector.tensor_copy(u[64:128, :], pvB[0:64, :])
                        rec = smp.tile([33, 512], F32, tag="rec")
                        nc.vector.reciprocal_approx_fast(rec[:], sums_col[:])
                        recb = smp.tile([33, 512], BF16, tag="recb")
                        nc.vector.tensor_copy(recb[:], rec[:])
                        bc2s = smp.tile([128, 512], BF16, tag="bc2s")
                        if GPB:
                            nc.gpsimd.partition_broadcast(bc2s[0:64, :],
                                                          recb[0:1, :])
                            nc.gpsimd.partition_broadcast(bc2s[64:128, :],
                                                          recb[32:33, :])
                        else:
                            bc2 = pspj.tile([128, 512], F32, tag="pacc",
                                            name="bc2")
                            nc.tensor.matmul(bc2[:], emat_sb[:], recb[:],
                                             start=True, stop=True)
                            nc.vector.tensor_copy(bc2s[:], bc2[:])
                        nc.vector.tensor_tensor(
                            attnT[:, q0:q0 + 512], u[:],
                            bc2s[:], op=ALU.mult)
                        nc.sync.dma_start(
                            ao_in[b][:].rearrange("(j p) t -> p j t", p=128)
                            [:, 2 * qc:2 * qc + 2, :],
                            attnT[:, q0:q0 + 512]
                            .rearrange("p (j t) -> p j t", j=2))
                        if after_qc is not None:
                            after_qc(qi)

                def b0_hook(qi):
                    if qi == 2:
                        unpack_b(1)

                emit_attention(0, after_qc=b0_hook, defer_kp=8)
                nc.gpsimd.collective_compute(
                    "AllToAll", ALU.bypass, replica_groups=RG,
                    ins=[ao_in[0][:].opt()], outs=[ao_out[0][:].opt()],
                )
                # batch-0 attention rows land during batch-1 attention
                aT0 = pjx.tile([128, NC * CH], BF16, name="aT0")
                proj0 = make_proj_slots(0, aT0)

                # emit aT0 loads lazily: they must come after b1's early
                # a2a DMAs on the SP queue, so stage them via slot thunks too.
                def load_aT(aTb, src):
                    def f():
                        a3 = aTb[:].rearrange("p (u t) -> p u t", u=NC)
                        s3 = src[:].rearrange("(u p) t -> p u t", p=128)
                        for r in range(0, NC, 2):
                            nc.sync.dma_start(a3[:, r:r + 2, :],
                                              s3[:, r:r + 2, :])
                    return f

                proj0.insert(0, load_aT(aT0, ao_out[0]))
                emit_attention(1, slots=proj0, slot_from_qc=1, qc_desc=True)
                nc.gpsimd.collective_compute(
                    "AllToAll", ALU.bypass, replica_groups=RG,
                    ins=[ao_in[1][:].opt()], outs=[ao_out[1][:].opt()],
                )
                # drain remaining batch-0 projection ops
                while proj0:
                    proj0.pop(0)()
                if DUM_TAIL:
                    for i in range(DUM_TAIL):
                        dt_ = pspj.tile([128, 512], F32, tag="pacc", name="dumtl")
                        nc.tensor.matmul(dt_[:], idn_sb[:],
                                         xn_sb[:, 512 * (i % 8):512 * (i % 8) + 512],
                                         start=True, stop=True)

                # batch-1 projection (straight emission)
                aT1 = pjx.tile([128, NC * CH], BF16, name="aT1")
                load_aT(aT1, ao_out[1])()
                proj1 = make_proj_slots(1, aT1)
                while proj1:
                    proj1.pop(0)()

    nc.compile()
    return nc


def host_prep(inputs):
    x = np.asarray(inputs["x"], np.float32)            # [B, T, DIM]
    ln_w = np.asarray(inputs["ln_w"], np.float32)
    ln_b = np.asarray(inputs["ln_b"], np.float32)
    qkv_w = np.asarray(inputs["qkv_w"], np.float32)
    qkv_b = np.asarray(inputs["qkv_b"], np.float32)
    proj_w = np.asarray(inputs["proj_w"], np.float32)
    proj_b = np.asarray(inputs["proj_b"], np.float32)

    # fold LN affine into qkv weights; fold 1/sqrt(D) into Q rows
    Wp = qkv_w * ln_w[None, :]
    bp = qkv_b + qkv_w @ ln_b
    Wp[0:DIM] *= D ** -0.5
    bp[0:DIM] *= D ** -0.5

    # row order per tensor: for dest core c, heads 2c, 2c+1
    def rows_for(blk):
        rr = []
        for c in range(NC):
            for h in (2 * c, 2 * c + 1):
                rr.extend(range(blk * DIM + h * D, blk * DIM + (h + 1) * D))
        return np.array(rr)

    q_rows, k_rows, v_rows = rows_for(0), rows_for(1), rows_for(2)

    def swz(wt):   # [rows, DIM] -> stationary layout [128, KT8 * rows]
        r = wt.shape[0]
        return np.ascontiguousarray(
            wt.T.reshape(KT8, 128, r).transpose(1, 0, 2).reshape(128, KT8 * r)
        ).astype(BF16_NP)

    wk = swz(Wp[k_rows])
    wq = swz(Wp[q_rows])
    wv = swz(Wp[v_rows])
    bk = np.ascontiguousarray(bp[k_rows].reshape(8, 128).T)
    bq = np.ascontiguousarray(bp[q_rows].reshape(8, 128).T)
    bv = np.ascontiguousarray(bp[v_rows].reshape(8, 128).T)

    idn = np.eye(128, dtype=np.float32).astype(BF16_NP)
    ones_r = np.ones((1, 128), BF16_NP)
    emat = np.zeros((33, 128), np.float32)
    emat[0, 0:64] = 1.0
    emat[32, 64:128] = 1.0
    emat = emat.astype(BF16_NP)
    pwt = swz(proj_w)
    pb = proj_b.reshape(1, DIM).astype(BF16_NP)

    xb = x.astype(BF16_NP)     # ship bf16
    in_maps = []
    for c in range(NC):
        # rows: [b0 tokens 256c:256c+256 ; b1 tokens 256c:256c+256]
        xc = np.concatenate([xb[0, CH * c:CH * (c + 1)],
                             xb[1, CH * c:CH * (c + 1)]], axis=0)
        in_maps.append(dict(
            x_c=np.ascontiguousarray(xc),
            wk=wk, wq=wq, wv=wv, bk=bk, bq=bq, bv=bv,
            pwt=pwt, pb=pb, idn=idn, ones_r=ones_r, emat=emat,
        ))
    return in_maps


_CACHED = {}


def kernel(**inputs) -> np.ndarray:
    _ensure_ntff_hook()
    from concourse import bass_utils
    if TRACE:
        bass_utils.upload_artifacts = lambda tmpdir: "/tmp/noupload"

    if "nc" not in _CACHED:
        _CACHED["nc"] = build_graph()
    nc = _CACHED["nc"]

    in_maps = host_prep(inputs)
    res = bass_utils.run_bass_kernel_spmd(
        nc, in_maps, core_ids=list(range(NC)), trace=TRACE,
        trace_cores=list(range(NC)) if TRACE else None)
    _CACHED["last_result"] = res
    out = np.empty((B, T, DIM), np.float32)
    for c in range(NC):
        oc = res.results[c]["out_c"]      # [512, 1024]
        out[0, CH * c:CH * (c + 1)] = oc[0:CH]
        out[1, CH * c:CH * (c + 1)] = oc[CH:2 * CH]
    return out


# revision 28
# speedup vs baseline: 1.1053x; 1.1053x over previous
"""Distributed Trainium2 Bass kernel for fused LayerNorm + causal multi-head
attention + output projection (B=2, T=2048, DIM=1024, H=16, D=64) on 8 cores.

Structure (v5):
  - Interleaved token ownership: core c owns tokens [256c:256c+256) of BOTH
    batches, so input/output redistribution splits per batch and pipelines.
  - Input side: three A2As ordered KQ-b0 (1MB, fires right after the
    firmware barrier), V (producer-transposed, softmax-ones columns
    embedded so the consumer DMAs straight into vnat layout), KQ-b1.
    Batch-0 attention starts while KQ-b1 is still in flight.
  - x shipped as bf16; weight loads split across many DMA engines in
    consumption order (wk interleaved with x, then wq, wv).
  - Attention: causal-trimmed score/exp/PV tiles, exp over both heads in
    one ACT instruction, diagonal-only masking, late normalization via a
    broadcast matmul, PV backlog of 2 to decouple PE from ACT; batch-0's
    first (largest) qc defers its first 8 PVs so scores/exp stream while
    the V collective lands.
  - Output: per-batch A2As; batch-0's projection matmuls are fed into
    batch-1 attention's PE bubbles (replacing keep-warm dummies with
    real work), batch-1 projection + split output DMAs form the tail.

Compute dtype: bf16 matmuls with fp32 PSUM accumulation.
LN affine params and the 1/sqrt(D) score scale are folded into the QKV
weights on the host.
"""
import os
import sys
import types
import numpy as np
import ml_dtypes

# ---------------------------------------------------------------- constants
B, T, DIM, D = 2, 2048, 1024, 64
H = DIM // D            # 16 heads
NC = 8                  # cores
TOK = B * T             # 4096 tokens
TPC = TOK // NC         # 512 tokens per core (256 per batch)
CH = TPC // 2           # 256-token per-batch chunk
KT8 = DIM // 128        # 8 contraction tiles
EPS = 1e-5

TRACE = bool(int(os.environ.get("BASS_KERNEL_TRACE", "0")))
DUM_LN = int(os.environ.get("DUM_LN", "40"))      # transposes during startup/LN
DUM_W1 = int(os.environ.get("DUM_W1", "60"))      # 512-wide, while waiting for qT
DUMF_W = int(os.environ.get("DUMF_W", "0"))     # width of b0 bubble fillers
DUM_TAIL = int(os.environ.get("DUM_TAIL", "36"))  # while A2A#5 flies
GPB = bool(int(os.environ.get("GPB", "0")))       # gpsimd partition_broadcast
SHARED_CC = bool(int(os.environ.get("SHARED_CC", "0")))

BF16_NP = ml_dtypes.bfloat16


def _ensure_ntff_hook():
    """The agent image lacks antenv.axon_hooks; recreate it so trace=True works."""
    if "antenv.axon_hooks" not in sys.modules:
        mod = types.ModuleType("antenv.axon_hooks")
        mod._hook = None
        def set_axon_ntff_profile_hook(h):
            mod._hook = h
        def get_axon_ntff_profile_hook():
            return mod._hook
        mod.set_axon_ntff_profile_hook = set_axon_ntff_profile_hook
        mod.get_axon_ntff_profile_hook = get_axon_ntff_profile_hook
        sys.modules["antenv.axon_hooks"] = mod
    m = sys.modules["antenv.axon_hooks"]
    if m.get_axon_ntff_profile_hook() is None:
        try:
            from trn_agent_boot.trn_boot import _ntff_profile_via_ctypes
            m.set_axon_ntff_profile_hook(
                _ntff_profile_via_ctypes("/opt/axon/libaxon_pjrt.so"))
        except Exception:
            pass


def build_graph():
    import concourse.bass as bass
    import concourse.bacc as bacc
    import concourse.tile as tile
    import concourse.mybir as mybir

    dt = mybir.dt
    F32, BF16, F8 = dt.float32, dt.bfloat16, dt.float8e4
    AF = mybir.ActivationFunctionType
    ALU = mybir.AluOpType
    RG = [list(range(NC))]
    SH = "Shared" if SHARED_CC else "Local"

    nc = bacc.Bacc(None, target_bir_lowering=False, debug=False, num_devices=NC)

    # ------------------------------------------------------------ I/O
    x_in = nc.dram_tensor("x_c", [TPC, DIM], BF16, kind="ExternalInput")
    wk_in = nc.dram_tensor("wk", [128, KT8 * 1024], BF16, kind="ExternalInput")
    wq_in = nc.dram_tensor("wq", [128, KT8 * 1024], BF16, kind="ExternalInput")
    wv_in = nc.dram_tensor("wv", [128, KT8 * 1024], BF16, kind="ExternalInput")
    bk_in = nc.dram_tensor("bk", [128, 8], F32, kind="ExternalInput")
    bq_in = nc.dram_tensor("bq", [128, 8], F32, kind="ExternalInput")
    bv_in = nc.dram_tensor("bv", [128, 8], F32, kind="ExternalInput")
    pwt_in = nc.dram_tensor("pwt", [128, KT8 * DIM], BF16, kind="ExternalInput")
    pb_in = nc.dram_tensor("pb", [1, DIM], BF16, kind="ExternalInput")
    idn_in = nc.dram_tensor("idn", [128, 128], BF16, kind="ExternalInput")
    ones_in = nc.dram_tensor("ones_r", [1, 128], BF16, kind="ExternalInput")
    emat_in = nc.dram_tensor("emat", [33, 128], BF16, kind="ExternalInput")
    out_dram = nc.dram_tensor("out_c", [TPC, DIM], F32, kind="ExternalOutput")

    with tile.TileContext(nc) as tc:
        with (
            tc.tile_pool(name="persist", bufs=1) as pers,
            tc.tile_pool(name="dram", bufs=1, space="DRAM") as dram,
        ):
            # ---------------- DRAM bounce buffers ----------------
            # A2A#1/#3: block c = [K 128 | Q 128] for dest c, one per batch
            kq_in = [dram.tile([NC * 256, CH], BF16, name=f"kq_in{b}")
                     for b in range(B)]
            kq_out = [dram.tile([NC * 256, CH], BF16, name=f"kq_out{b}")
                      for b in range(B)]
            # A2A#2: V, block c = [128 local tokens, 4 x (A 64|1|B 64|1)]
            v_in = dram.tile([NC * 128, 520], BF16)
            v_out = dram.tile([NC * 128, 520], BF16)
            ao_in = [dram.tile([NC * 128, CH], BF16, name=f"ao_in{b}")
                     for b in range(B)]
            ao_out = [dram.tile([NC * 128, CH], BF16, addr_space=SH,
                                name=f"ao_out{b}") for b in range(B)]

            if GPB:
                from concourse import library_config
                nc.gpsimd.load_library(library_config.attn)

            # idn first: transposes + dummies need it early; it is tiny
            idn_sb = pers.tile([128, 128], BF16)
            nc.sync.dma_start(idn_sb[:], idn_in[:])

            # x tiles FIRST (LN critical path), then QKV weights in
            # consumption order (wk, wq, wv) split across many DMA engines.
            xts = []
            with tc.tile_pool(name="ln_x", bufs=1) as lnx:
                wk_sb = pers.tile([128, KT8 * 1024], BF16)
                for t in range(4):
                    xt = lnx.tile([128, DIM], BF16, tag=f"xt{t}", name=f"xt{t}")
                    for hh in range(2):
                        nc.sync.dma_start(
                            xt[:, 512 * hh:512 * (hh + 1)],
                            x_in[128 * t:128 * (t + 1), 512 * hh:512 * (hh + 1)])
                    xts.append(xt)
                    for i in (2 * t, 2 * t + 1):
                        nc.sync.dma_start(wk_sb[:, 1024 * i:1024 * (i + 1)],
                                          wk_in[:, 1024 * i:1024 * (i + 1)])
                wq_sb = pers.tile([128, KT8 * 1024], BF16)
                for i in range(8):
                    nc.sync.dma_start(wq_sb[:, 1024 * i:1024 * (i + 1)],
                                      wq_in[:, 1024 * i:1024 * (i + 1)])
                wv_sb = pers.tile([128, KT8 * 1024], BF16)
                for i in range(8):
                    nc.sync.dma_start(wv_sb[:, 1024 * i:1024 * (i + 1)],
                                      wv_in[:, 1024 * i:1024 * (i + 1)])
                bk_sb = pers.tile([128, 8], F32)
                nc.sync.dma_start(bk_sb[:], bk_in[:])
                bq_sb = pers.tile([128, 8], F32)
                nc.sync.dma_start(bq_sb[:], bq_in[:])
                bv_sb = pers.tile([128, 8], F32)
                nc.sync.dma_start(bv_sb[:], bv_in[:])

                # ============= P1: LayerNorm (token slice, natural) ========
                xn_sb = pers.tile([128, 4 * DIM], BF16)
                with tc.tile_pool(name="ln", bufs=4) as lnp:
                    if DUM_LN:
                        with tc.tile_pool(name="ps_dln", bufs=1,
                                          space="PSUM") as psdl:
                            dln = psdl.tile([128, 128], BF16, tag="dln")
                            for i in range(DUM_LN):
                                nc.tensor.transpose(dln[:], idn_sb[:], idn_sb[:])
                    for t in range(4):
                        xt = xts[t]
                        stats = lnp.tile([128, 12], F32, tag="stats")
                        nc.vector.bn_stats(stats[:, 0:6], xt[:, 0:512])
                        nc.vector.bn_stats(stats[:, 6:12], xt[:, 512:1024])
                        mv = lnp.tile([128, 2], F32, tag="mv")
                        nc.vector.bn_aggr(mv[:], stats[:])
                        vareps = lnp.tile([128, 1], F32, tag="vareps")
                        nc.vector.tensor_scalar(vareps[:], mv[:, 1:2], 1.0, EPS,
                                                op0=ALU.mult, op1=ALU.add)
                        nmu = lnp.tile([128, 1], F32, tag="nmu")
                        nc.vector.tensor_scalar_mul(nmu[:], mv[:, 0:1], -1.0)
                        std = lnp.tile([128, 1], F32, tag="std")
                        nc.scalar.activation(std[:], vareps[:], AF.Sqrt)
                        rstd = lnp.tile([128, 1], F32, tag="rstd")
                        nc.vector.reciprocal(rstd[:], std[:])
                        nmr = lnp.tile([128, 1], F32, tag="nmr")
                        nc.vector.scalar_tensor_tensor(
                            nmr[:], nmu[:], 1.0, rstd[:],
                            op0=ALU.mult, op1=ALU.mult)
                        nc.scalar.activation(
                            xn_sb[:, DIM * t:DIM * (t + 1)], xt[:],
                            AF.Identity, bias=nmr[:], scale=rstd[:])

            # preload the EXP activation table while nothing else uses ACT
            exq = pers.tile([1, 1], F32, name="exq")
            nc.scalar.activation(exq[:], xn_sb[0:1, 0:1], AF.Exp)

            # ================= P2: transpose xn -> xnT =====================
            xnT_sb = pers.tile([128, KT8 * TPC], BF16)  # [dim-tile part, k*512+t]
            with tc.tile_pool(name="ps_tr", bufs=6, space="PSUM") as pstr:
                for t in range(4):
                    for k in range(KT8):
                        trp = pstr.tile([128, 128], BF16, tag="tr")
                        nc.tensor.transpose(
                            trp[:], xn_sb[:, DIM * t + 128 * k: DIM * t + 128 * (k + 1)],
                            idn_sb[:])
                        nc.vector.tensor_copy(
                            xnT_sb[:, TPC * k + 128 * t: TPC * k + 128 * (t + 1)],
                            trp[:])

            # ================= P3: K, Q, V on own tokens + 3 A2As ==========
            wk3 = wk_sb[:].rearrange("p (k r) -> p k r", r=1024)
            wq3 = wq_sb[:].rearrange("p (k r) -> p k r", r=1024)
            wv3 = wv_sb[:].rearrange("p (k r) -> p k r", r=1024)
            with (
                tc.tile_pool(name="ps_qkv", bufs=3, space="PSUM") as psq,
                tc.tile_pool(name="ps_vt", bufs=2, space="PSUM") as psvt,
                tc.tile_pool(name="ps_dum", bufs=1, space="PSUM") as psd,
                tc.tile_pool(name="stg", bufs=4) as stg,
            ):
                def emit_half(w3, g, bias_sb, b, dst, use_act=False):
                    # one K/Q group, batch-b column half only
                    psgf = psq.tile([128, TPC], F32, tag="qg", name="psgf")
                    psg = psgf[:, 0:CH]
                    for k in range(KT8):
                        nc.tensor.matmul(
                            psg, w3[:, k, 128 * g:128 * (g + 1)],
                            xnT_sb[:, TPC * k + CH * b:TPC * k + CH * (b + 1)],
                            start=(k == 0), stop=(k == KT8 - 1))
                    st = stg.tile([128, CH], BF16, tag="st", name="st")
                    if use_act:
                        # Scalar engine: keeps pass-1 staging off the DVE
                        # queue, whose sem-recycle guards wait on the unpack
                        nc.scalar.activation(st[:], psg, AF.Identity,
                                             bias=bias_sb[:, g:g + 1])
                    else:
                        nc.vector.tensor_scalar(
                            st[:], psg, bias_sb[:, g:g + 1], None, op0=ALU.add)
                    nc.sync.dma_start(dst, st[:])

                # pass 0: K+Q batch-0 columns -> A2A#1 fires first
                for g in range(8):
                    emit_half(wk3, g, bk_sb, 0,
                              kq_in[0][256 * g:256 * g + 128, :])
                for g in range(8):
                    emit_half(wq3, g, bq_sb, 0,
                              kq_in[0][256 * g + 128:256 * g + 256, :])
                nc.gpsimd.collective_compute(
                    "AllToAll", ALU.bypass, replica_groups=RG,
                    ins=[kq_in[0][:].opt()], outs=[kq_out[0][:].opt()],
                )
                # V (full tokens): producer-side transpose with ones
                for g in range(8):
                    psg = psq.tile([128, TPC], F32, tag="qg", name="psgv")
                    for k in range(KT8):
                        nc.tensor.matmul(
                            psg[:], wv3[:, k, 128 * g:128 * (g + 1)],
                            xnT_sb[:, TPC * k:TPC * (k + 1)],
                            start=(k == 0), stop=(k == KT8 - 1))
                    st = stg.tile([128, TPC], BF16, tag="st2", name="stv")
                    nc.vector.tensor_scalar(
                        st[:], psg[:], bv_sb[:, g:g + 1], None, op0=ALU.add)
                    vtp = psvt.tile([128, 512], BF16, tag="vtp")
                    for i in range(4):
                        nc.tensor.transpose(
                            vtp[:, 128 * i:128 * (i + 1)],
                            st[:, 128 * i:128 * (i + 1)], idn_sb[:])
                    vts = stg.tile([128, 520], BF16, tag="vts", name="vts")
                    v4s = vts[:].rearrange("p (i a w) -> p i a w", i=4, w=65)
                    nc.vector.memset(v4s[:, :, :, 64:65], 1.0)
                    nc.vector.tensor_copy(
                        v4s[:, :, :, 0:64],
                        vtp[:].rearrange("p (i a w) -> p i a w", i=4, w=64))
                    nc.sync.dma_start(v_in[128 * g:128 * (g + 1), :], vts[:])
                nc.gpsimd.collective_compute(
                    "AllToAll", ALU.bypass, replica_groups=RG,
                    ins=[v_in[:].opt()], outs=[v_out[:].opt()],
                )
                # pass 1: K+Q batch-1 columns (bias on Scalar engine)
                for g in range(8):
                    emit_half(wk3, g, bk_sb, 1,
                              kq_in[1][256 * g:256 * g + 128, :], use_act=True)
                for g in range(8):
                    emit_half(wq3, g, bq_sb, 1,
                              kq_in[1][256 * g + 128:256 * g + 256, :],
                              use_act=True)
                nc.gpsimd.collective_compute(
                    "AllToAll", ALU.bypass, replica_groups=RG,
                    ins=[kq_in[1][:].opt()], outs=[kq_out[1][:].opt()],
                )

                # deferred weight loads (needed only from attention onwards)
                pwt_sb = pers.tile([128, KT8 * DIM], BF16)
                for i in range(4):
                    nc.sync.dma_start(pwt_sb[:, 2048 * i:2048 * (i + 1)],
                                      pwt_in[:, 2048 * i:2048 * (i + 1)])
                pb_sb = pers.tile([1, DIM], BF16)
                nc.sync.dma_start(pb_sb[:], pb_in[:])
                ones_sb = pers.tile([1, 128], BF16)
                nc.sync.dma_start(ones_sb[:], ones_in[:])
                emat_sb = pers.tile([33, 128], BF16)
                nc.sync.dma_start(emat_sb[:], emat_in[:])

                sums_col = pers.tile([33, 512], F32)
                nc.vector.memset(sums_col[:], 1.0)

                # ---- unpack: kT/qT uninterleave, per batch as A2As land
                kqT = pers.tile([128, 2 * TOK], BF16)
                kT_sb = kqT[:, 0:TOK]
                qT_sb = kqT[:, TOK:2 * TOK]
                kqT5 = kqT[:].rearrange("p (s j u t) -> p s j u t",
                                        s=2, j=2, u=NC)
                vnat = []
                for b in range(B):
                    vb = pers.tile([128, 16 * 130], BF16, name=f"vnat{b}")
                    vnat.append(vb)

                v4o = v_out[:].rearrange("(u p) (bb c) -> p u bb c",
                                         p=128, bb=2)

                def unpack_b(b):
                    kq4 = kq_out[b][:].rearrange("(u s p) t -> p u s t",
                                                 p=128, s=2)
                    # b0 runs qc3 first: its queries live in chunks r=6,7
                    order = (6, 7, 0, 1, 2, 3, 4, 5) if b == 0 else range(NC)
                    for r in order:
                        nc.sync.dma_start(kqT5[:, :, b, r, :],
                                          kq4[:, r, :, :])
                    vn3 = vnat[b][:].rearrange("p (u c) -> p u c", u=NC)
                    for r in range(NC):
                        nc.sync.dma_start(vn3[:, r, :], v4o[:, r, b, :])

                unpack_b(0)

                # keep-warm while waiting for qT
                if DUM_W1:
                    dum = psd.tile([128, 512], F32, tag="dum")
                    for i in range(DUM_W1):
                        nc.tensor.matmul(dum[:], idn_sb[:],
                                         xn_sb[:, 512 * (i % 8):512 * (i % 8) + 512],
                                         start=True, stop=True)

            attnT = pers.tile([128, TOK], BF16)

            # ================= P6: attention + overlapped projection =======
            with (
                tc.tile_pool(name="pt", bufs=10) as ptp,
                tc.tile_pool(name="un", bufs=4) as unp,
                tc.tile_pool(name="ps_s", bufs=2, space="PSUM") as pss,
                tc.tile_pool(name="ps_pv", bufs=2, space="PSUM") as psp,
                tc.tile_pool(name="ps_pj", bufs=2, space="PSUM") as pspj,
                tc.tile_pool(name="sm", bufs=2) as smp,
                tc.tile_pool(name="pjx", bufs=1) as pjx,
                tc.tile_pool(name="pjo", bufs=2) as pjo,
            ):
                pwt3 = pwt_sb[:].rearrange("p (k o) -> p k o", o=DIM)

                def dum_slot(n=1, w=DUMF_W):
                    # keep-warm into the (currently free) proj psum pool
                    for _ in range(n):
                        dt_ = pspj.tile([128, 512], F32, tag="pacc", name="dumpj")
                        nc.tensor.matmul(dt_[:, 0:w], idn_sb[:],
                                         xn_sb[:, 0:w], start=True, stop=True)

                def make_proj_slots(b, aTb):
                    """Thunk list: projection of batch b fed one op per slot."""
                    slots = []
                    state = {}

                    def mm(m, ck):
                        def f():
                            if ck == 0:
                                state["acc"] = pspj.tile([128, 512], F32, tag="pacc", name="pacc")
                            tt, h = m // 2, m % 2
                            nc.tensor.matmul(
                                state["acc"][:],
                                aTb[:, 256 * ck + 128 * tt:256 * ck + 128 * (tt + 1)],
                                pwt3[:, ck, 512 * h:512 * (h + 1)],
                                start=(ck == 0), stop=False)
                        return f

                    def fin(m):
                        def f():
                            tt, h = m // 2, m % 2
                            acc = state["acc"]
                            nc.tensor.matmul(acc[:], ones_sb[0:1, :],
                                             pb_sb[:, 512 * h:512 * (h + 1)],
                                             start=False, stop=True)
                            ot = pjo.tile([128, 512], F32, tag="ot", name="ot")
                            nc.vector.tensor_copy(ot[:], acc[:])
                            r0 = 256 * b + 128 * tt
                            nc.sync.dma_start(
                                out_dram[r0:r0 + 64, 512 * h:512 * (h + 1)],
                                ot[0:64, :])
                            nc.sync.dma_start(
                                out_dram[r0 + 64:r0 + 128, 512 * h:512 * (h + 1)],
                                ot[64:128, :])
                        return f

                    for m in range(4):
                        for ck in range(KT8):
                            slots.append(mm(m, ck))
                        slots.append(fin(m))
                    return slots

                def emit_attention(b, slots=None, slot_from_qc=99,
                                   after_qc=None, defer_kp=0, qc_desc=False):
                    def slot(n):
                        for _ in range(n):
                            if slots and qi >= slot_from_qc:
                                if len(slots):
                                    slots.pop(0)()
                            elif DUMF_W:
                                dum_slot(1)

                    def emit_pv(pkp, pp, pc0, stop):
                        nc.tensor.matmul(pvA[:, pc0:512],
                                         vnat[b][:, 130 * pkp:130 * pkp + 65],
                                         pp[:, pc0:512],
                                         start=(pkp == 0), stop=stop)
                        nc.tensor.matmul(pvB[:, pc0:512],
                                         vnat[b][:, 130 * pkp + 65:130 * pkp + 130],
                                         pp[:, 512 + pc0:1024],
                                         start=(pkp == 0), stop=stop)

                    for qi, qc in enumerate((3, 2, 1, 0)
                                            if (defer_kp or qc_desc) else
                                            (0, 1, 2, 3)):
                        q0 = b * T + 512 * qc
                        pvA = psp.tile([65, 512], F32, tag="pv")
                        pvB = psp.tile([65, 512], F32, tag="pv")
                        nkp = 4 * qc + 4
                        D = defer_kp if qi == 0 else 0
                        pvq = []
                        for kp in range(nkp):
                            k0 = b * T + 128 * kp
                            c0 = max(0, 128 * (kp - 4 * qc))
                            sAB = pss.tile([128, 1024], F32, tag="s")
                            nc.tensor.matmul(sAB[:, c0:512],
                                             kT_sb[0:64, k0:k0 + 128],
                                             qT_sb[0:64, q0 + c0:q0 + 512],
                                             start=True, stop=True)
                            nc.tensor.matmul(sAB[:, 512 + c0:1024],
                                             kT_sb[64:128, k0:k0 + 128],
                                             qT_sb[64:128, q0 + c0:q0 + 512],
                                             start=True, stop=True)
                            ne = 0
                            nmax = 3 if len(pvq) > 4 else 2
                            while kp >= D and len(pvq) > 2 and ne < nmax:
                                pk, pp_, pc = pvq.pop(0)
                                emit_pv(pk, pp_, pc, False)
                                ne += 1
                            if ne == 0:
                                slot(2)
                            pAB = ptp.tile([128, 1024], BF16, tag="pA")
                            if c0 == 0:
                                nc.scalar.activation(pAB[:], sAB[:], AF.Exp)
                            else:
                                s3 = sAB[:].rearrange("p (j c) -> p j c", j=2)
                                p3 = pAB[:].rearrange("p (j c) -> p j c", j=2)
                                nc.scalar.activation(p3[:, :, c0:512],
                                                     s3[:, :, c0:512], AF.Exp)
                            if kp >= 4 * qc:
                                p3 = pAB[:].rearrange("p (j c) -> p j c", j=2)
                                nc.gpsimd.affine_select(
                                    p3[:, :, c0:c0 + 128], p3[:, :, c0:c0 + 128],
                                    pattern=[[0, 2], [1, 128]],
                                    compare_op=ALU.is_ge, fill=0.0,
                                    base=0, channel_multiplier=-1)
                            if kp + 1 < nkp:
                                slot(1)
                            pvq.append((kp, pAB, c0))
                        while pvq:
                            pk, pp_, pc = pvq.pop(0)
                            emit_pv(pk, pp_, pc, stop=(not pvq))
                        # late normalization: copy PV out of PSUM immediately
                        u = unp.tile([128, 512], BF16, tag="u")
                        nc.vector.tensor_copy(sums_col[0:1, :], pvA[64:65, :])
                        nc.vector.tensor_copy(sums_col[32:33, :], pvB[64:65, :])
                        nc.vector.tensor_copy(u[0:64, :], pvA[0:64, :])
                        nc.vector.tensor_copy(u[64:128, :], pvB[0:64, :])
                        rec = smp.tile([33, 512], F32, tag="rec")
                        nc.vector.reciprocal_approx_fast(rec[:], sums_col[:])
                        recb = smp.tile([33, 512], BF16, tag="recb")
                        nc.vector.tensor_copy(recb[:], rec[:])
                        bc2s = smp.tile([128, 512], BF16, tag="bc2s")
                        if GPB:
                            nc.gpsimd.partition_broadcast(bc2s[0:64, :],
                                                          recb[0:1, :])
                            nc.gpsimd.partition_broadcast(bc2s[64:128, :],
                                                          recb[32:33, :])
                        else:
                            bc2 = pspj.tile([128, 512], F32, tag="pacc",
                                            name="bc2")
                            nc.tensor.matmul(bc2[:], emat_sb[:], recb[:],
                                             start=True, stop=True)
                            nc.vector.tensor_copy(bc2s[:], bc2[:])
                        nc.vector.tensor_tensor(
                            attnT[:, q0:q0 + 512], u[:],
                            bc2s[:], op=ALU.mult)
                        nc.sync.dma_start(
                            ao_in[b][:].rearrange("(j p) t -> p j t", p=128)
                            [:, 2 * qc:2 * qc + 2, :],
                            attnT[:, q0:q0 + 512]
                            .rearrange("p (j t) -> p j t", j=2))
                        if after_qc is not None:
                            after_qc(qi)

                def b0_hook(qi):
                    if qi == 2:
                        unpack_b(1)

                emit_attention(0, after_qc=b0_hook, defer_kp=8)
                nc.gpsimd.collective_compute(
                    "AllToAll", ALU.bypass, replica_groups=RG,
                    ins=[ao_in[0][:].opt()], outs=[ao_out[0][:].opt()],
                )
                # batch-0 attention rows land during batch-1 attention
                aT0 = pjx.tile([128, NC * CH], BF16, name="aT0")
                proj0 = make_proj_slots(0, aT0)

                # emit aT0 loads lazily: they must come after b1's early
                # a2a DMAs on the SP queue, so stage them via slot thunks too.
                def load_aT(aTb, src):
                    def f():
                        a3 = aTb[:].rearrange("p (u t) -> p u t", u=NC)
                        s3 = src[:].rearrange("(u p) t -> p u t", p=128)
                        for r in range(0, NC, 2):
                            nc.sync.dma_start(a3[:, r:r + 2, :],
                                              s3[:, r:r + 2, :])
                    return f

                proj0.insert(0, load_aT(aT0, ao_out[0]))
                emit_attention(1, slots=proj0, slot_from_qc=1, qc_desc=True)
                nc.gpsimd.collective_compute(
                    "AllToAll", ALU.bypass, replica_groups=RG,
                    ins=[ao_in[1][:].opt()], outs=[ao_out[1][:].opt()],
                )
                # drain remaining batch-0 projection ops
                while proj0:
                    proj0.pop(0)()
                if DUM_TAIL:
                    for i in range(DUM_TAIL):
                        dt_ = pspj.tile([128, 512], F32, tag="pacc", name="dumtl")
                        nc.tensor.matmul(dt_[:], idn_sb[:],
                                         xn_sb[:, 512 * (i % 8):512 * (i % 8) + 512],
                                         start=True, stop=True)

                # batch-1 projection (straight emission)
                aT1 = pjx.tile([128, NC * CH], BF16, name="aT1")
                load_aT(aT1, ao_out[1])()
                proj1 = make_proj_slots(1, aT1)
                while proj1:
                    proj1.pop(0)()

    nc.compile()
    return nc


def host_prep(inputs):
    x = np.asarray(inputs["x"], np.float32)            # [B, T, DIM]
    ln_w = np.asarray(inputs["ln_w"], np.float32)
    ln_b = np.asarray(inputs["ln_b"], np.float32)
    qkv_w = np.asarray(inputs["qkv_w"], np.float32)
    qkv_b = np.asarray(inputs["qkv_b"], np.float32)
    proj_w = np.asarray(inputs["proj_w"], np.float32)
    proj_b = np.asarray(inputs["proj_b"], np.float32)

    # fold LN affine into qkv weights; fold 1/sqrt(D) into Q rows
    Wp = qkv_w * ln_w[None, :]
    bp = qkv_b + qkv_w @ ln_b
    Wp[0:DIM] *= D ** -0.5
    bp[0:DIM] *= D ** -0.5

    # row order per tensor: for dest core c, heads 2c, 2c+1
    def rows_for(blk):
        rr = []
        for c in range(NC):
            for h in (2 * c, 2 * c + 1):
                rr.extend(range(blk * DIM + h * D, blk * DIM + (h + 1) * D))
        return np.array(rr)

    q_rows, k_rows, v_rows = rows_for(0), rows_for(1), rows_for(2)

    def swz(wt):   # [rows, DIM] -> stationary layout [128, KT8 * rows]
        r = wt.shape[0]
        return np.ascontiguousarray(
            wt.T.reshape(KT8, 128, r).transpose(1, 0, 2).reshape(128, KT8 * r)
        ).astype(BF16_NP)

    wk = swz(Wp[k_rows])
    wq = swz(Wp[q_rows])
    wv = swz(Wp[v_rows])
    bk = np.ascontiguousarray(bp[k_rows].reshape(8, 128).T)
    bq = np.ascontiguousarray(bp[q_rows].reshape(8, 128).T)
    bv = np.ascontiguousarray(bp[v_rows].reshape(8, 128).T)

    idn = np.eye(128, dtype=np.float32).astype(BF16_NP)
    ones_r = np.ones((1, 128), BF16_NP)
    emat = np.zeros((33, 128), np.float32)
    emat[0, 0:64] = 1.0
    emat[32, 64:128] = 1.0
    emat = emat.astype(BF16_NP)
    pwt = swz(proj_w)
    pb = proj_b.reshape(1, DIM).astype(BF16_NP)

    xb = x.astype(BF16_NP)     # ship bf16
    in_maps = []
    for c in range(NC):
        # rows: [b0 tokens 256c:256c+256 ; b1 tokens 256c:256c+256]
        xc = np.concatenate([xb[0, CH * c:CH * (c + 1)],
                             xb[1, CH * c:CH * (c + 1)]], axis=0)
        in_maps.append(dict(
            x_c=np.ascontiguousarray(xc),
            wk=wk, wq=wq, wv=wv, bk=bk, bq=bq, bv=bv,
            pwt=pwt, pb=pb, idn=idn, ones_r=ones_r, emat=emat,
        ))
    return in_maps


_CACHED = {}


def kernel(**inputs) -> np.ndarray:
    _ensure_ntff_hook()
    from concourse import bass_utils
    if TRACE:
        bass_utils.upload_artifacts = lambda tmpdir: "/tmp/noupload"

    if "nc" not in _CACHED:
        _CACHED["nc"] = build_graph()
    nc = _CACHED["nc"]

    in_maps = host_prep(inputs)
    res = bass_utils.run_bass_kernel_spmd(
        nc, in_maps, core_ids=list(range(NC)), trace=TRACE,
        trace_cores=list(range(NC)) if TRACE else None)
    _CACHED["last_result"] = res
    out = np.empty((B, T, DIM), np.float32)
    for c in range(NC):
        oc = res.results[c]["out_c"]      # [512, 1024]
        out[0, CH * c:CH * (c + 1)] = oc[0:CH]
        out[1, CH * c:CH * (c + 1)] = oc[CH:2 * CH]
    return out
